# revision 3
# baseline (speedup 1.0000x reference)
"""GEMV kernel for Trainium2: out = x @ W.T + b, sharded over 8 NeuronCores.

Shapes (hardcoded): x [1, 147456] f32, W [1000, 147456] f32, b [1000] f32.

Sharding: k-parallel (input-dim). Core c takes k-slice [c*18432, (c+1)*18432)
of ALL 1000 classes; each core produces a partial [1000] dot-product vector
over its k-slice, and the host sums the 8 partials (+bias) — a 16K-flop
unshard vs 147M device MACs.

Per-core strategy (memory-bound -> minimize HBM bytes, PE does the math):
  - W is ternary {-1,0,1}: exactly representable in fp8e4 (e4m3), quartering
    HBM traffic vs f32 (73.7 MB -> 18.9 MB per core, incl. 1000->1024 class
    padding).
  - x is split x = x_hi + x_lo with both parts in fp8e4 (x_hi = fp8(x),
    x_lo = fp8(x - x_hi)); measured rel err 7e-4, far under the 2e-2 gate.
  - Layout: per k-chunk j (128 k values), W^T block [128 k-part, 1024
    classes] fp8; 144 chunks grouped 9-per-DMA (16 DMAs x 1.18 MB/exec,
    line-rate descriptors), tile pool bufs=17 so the DMA stream runs ahead
    of compute.
  - PE: per chunk, lhsT = x2[:, 2j:2j+2] (the [128,2] hi/lo stationary),
    rhs = the W^T block, accumulated over all 144 chunks into psum[2, 512]
    x 2 banks (start at j=0, stop at j=143). 288 back-to-back N=512 fp8
    matmuls per exec keep the PE warm; DMA (53 us floor) and PE (~60 us)
    are roughly balanced.
  - Tail per exec: DVE copies psum [2,512]x2 -> SBUF [2,1024], DMA out.
    Host sums the hi/lo rows together with the cross-core reduce.
  - loop_n repeats the whole computation in-NEFF (each exec re-reads all
    of W from HBM) so per-dispatch overhead amortizes when timing.
"""

import numpy as np
import ml_dtypes

import concourse.bacc as bacc
import concourse.mybir as mybir
import concourse.tile as tile
from concourse.bass_utils import run_bass_kernel_spmd

N_CORES = 8
N_CLASSES = 1000
N_IN = 147456
P = 128                       # SBUF partitions / k-chunk size
KS = N_IN // N_CORES          # 18432 k values per core
NCH = KS // P                 # 144 k-chunks per core
NCLP = 1024                   # classes padded
NB = 2                        # psum banks (512 classes each)
GCH = 9                       # chunks per DMA group
NG = NCH // GCH               # 16 DMA groups
GW = GCH * NCLP               # 9216 free bytes per group tile

F8 = ml_dtypes.float8_e4m3

_prog_cache = {}


def _build_program(loop_n=1):
    if loop_n in _prog_cache:
        return _prog_cache[loop_n]

    nc = bacc.Bacc("TRN2", target_bir_lowering=False, debug=False, num_devices=N_CORES)
    f32 = mybir.dt.float32
    f8 = mybir.dt.float8e4
    x_d = nc.dram_tensor("x2", [P, 2 * NCH], f8, kind="ExternalInput")
    w_d = nc.dram_tensor("W", [NG, P, GW], f8, kind="ExternalInput")
    o_d = nc.dram_tensor("out", [2, NCLP], f32, kind="ExternalOutput")

    with tile.TileContext(nc) as tc:
        with (
            tc.tile_pool(name="xpool", bufs=1) as xpool,
            tc.tile_pool(name="wpool", bufs=NG + 1) as wpool,
            tc.tile_pool(name="opool", bufs=2) as opool,
            tc.tile_pool(name="psum", bufs=2, space="PSUM") as psum_pool,
        ):
            x_t = xpool.tile([P, 2 * NCH], f8)
            nc.sync.dma_start(x_t[:], x_d[:])

            for _ in range(loop_n):
                ps0 = psum_pool.tile([2, 512], f32, tag="ps0")
                ps1 = psum_pool.tile([2, 512], f32, tag="ps1")
                ps = [ps0, ps1]
                for g in range(NG):
                    w_t = wpool.tile([P, GW], f8, tag="w")
                    nc.sync.dma_start(w_t[:], w_d[g])
                    for jj in range(GCH):
                        j = g * GCH + jj
                        lhsT = x_t[:, 2 * j : 2 * j + 2]
                        for b in range(NB):
                            rhs = w_t[:, jj * NCLP + b * 512 : jj * NCLP + (b + 1) * 512]
                            nc.tensor.matmul(
                                ps[b][:],
                                lhsT,
                                rhs,
                                start=(j == 0),
                                stop=(j == NCH - 1),
                            )
                ot = opool.tile([2, NCLP], f32, tag="o")
                for b in range(NB):
                    nc.vector.tensor_copy(ot[:, b * 512 : (b + 1) * 512], ps[b][:])
                nc.sync.dma_start(o_d[:], ot[:])

    nc.finalize()
    _prog_cache[loop_n] = nc
    return nc


def _in_maps(x, W, b):
    x_flat = np.asarray(x, dtype=np.float32).reshape(N_IN)
    W32 = np.asarray(W, dtype=np.float32)
    in_maps = []
    for c in range(N_CORES):
        ks = slice(c * KS, (c + 1) * KS)
        xs = x_flat[ks].reshape(NCH, P)
        hi = xs.astype(F8)
        lo = (xs - hi.astype(np.float32)).astype(F8)
        x2 = np.stack([hi, lo], axis=-1).transpose(1, 0, 2).reshape(P, 2 * NCH)
        # W block: [1000, 18432] -> fp8 -> [NCH, P, 1024] -> grouped [NG, P, GW]
        wc = W32[:, ks].astype(F8)
        a = np.zeros((NCH, P, NCLP), dtype=F8)
        a[:, :, :N_CLASSES] = wc.reshape(N_CLASSES, NCH, P).transpose(1, 2, 0)
        wg = np.ascontiguousarray(
            a.reshape(NG, GCH, P, NCLP).transpose(0, 2, 1, 3).reshape(NG, P, GW)
        )
        in_maps.append({"x2": np.ascontiguousarray(x2), "W": wg})
    return in_maps


def _run(x, W, b, trace=False, loop_n=1, **kwargs):
    nc = _build_program(loop_n)
    in_maps = _in_maps(x, W, b)
    return run_bass_kernel_spmd(nc, in_maps, list(range(N_CORES)), trace=trace, **kwargs)


def kernel(x, W, b):
    res = _run(x, W, b)
    acc = np.zeros(NCLP, dtype=np.float32)
    for r in res.results:
        o = np.asarray(r["out"], dtype=np.float32)
        acc += o[0] + o[1]
    out = acc[:N_CLASSES] + np.asarray(b, dtype=np.float32)
    return out.reshape(1, N_CLASSES).astype(np.float32)


# revision 4
# speedup vs baseline: 1.3708x; 1.3708x over previous
"""GEMV kernel for Trainium2: out = x @ W.T + b, sharded over 8 NeuronCores.

Shapes (hardcoded): x [1, 147456] f32, W [1000, 147456] f32, b [1000] f32.

Sharding: k-parallel (input-dim). Core c takes k-slice [c*18432, (c+1)*18432)
of ALL 1000 classes; each core produces partial dot products over its
k-slice and the host sums the 8 partials (+bias) — a 16K-flop unshard vs
147M device MACs.

Per-core strategy (memory-bound -> minimize HBM bytes; PE does the math):
  - W is ternary {-1,0,1}: exactly representable in fp8e4 (e4m3), quartering
    HBM traffic vs f32 (73.7 MB -> 18.9 MB per core incl. class padding).
  - x is split x = x_hi + x_lo, both fp8e4 (x_hi = fp8(x), x_lo =
    fp8(x - x_hi)); measured rel err 7.2e-4, far under the 2e-2 gate.
  - PE matmuls run perf_mode=DoubleRow: each cell holds 2 fp8 weights, so
    one matmul contracts 256 k-values and streams W at 2 elem/cycle/
    partition. Per exec: 72 chunks x 2 psum banks = 144 N=512 matmuls
    accumulated in PSUM (start at chunk 0, stop at chunk 71), ~37 us PE —
    under the ~52 us HBM floor for 18.9 MB, so the kernel is DMA-bound.
  - lhsT (stationary x hi/lo pair) is a 3D AP [128, 2, 2], pair dim padded
    to stride 16 (LDWEIGHTS DoubleRow wants step%16==0). rhs (moving W
    block) is [128, 2, 512] with pair stride NCLP.
  - W layout: per 256-k chunk, a [128 part, 2, 1024 cls] fp8 block; chunks
    grouped 6-per-DMA (12 x 1.57 MB line-rate DMAs per exec) with a
    13-slot tile pool so the DMA stream runs ahead of the PE.
  - Tail per exec: DVE copies psum[2,512]x2 -> SBUF, out-DMA on the scalar
    HWDGE ring (keeps the W-load ring clean). Host sums hi/lo rows during
    the cross-core reduce.
  - loop_n repeats the whole computation in-NEFF (each exec re-reads all of
    W from HBM). There is a ~4 ms fixed per-NEFF-call dispatch cost in this
    environment that does NOT overlap device execution, so timing uses a
    large loop_n (1024) to amortize it; compile is fast (~5 s per 256
    execs).
"""

import numpy as np
import ml_dtypes

import concourse.bacc as bacc
import concourse.mybir as mybir
import concourse.tile as tile
from concourse.bass_utils import run_bass_kernel_spmd

N_CORES = 8
N_CLASSES = 1000
N_IN = 147456
P = 128                       # SBUF partitions
KS = N_IN // N_CORES          # 18432 k values per core
CH = 2 * P                    # 256 k per DoubleRow chunk
NCH = KS // CH                # 72 chunks per core
NCLP = 1024                   # classes padded
NB = 2                        # psum banks (512 classes each)
GCH = 6                       # chunks per DMA group
NG = NCH // GCH               # 12 DMA groups
GW = GCH * 2 * NCLP           # 12288 free bytes per group tile
XP = 16                       # x pair-stride padding (LDW step%16==0)

F8 = ml_dtypes.float8_e4m3

_prog_cache = {}


def _build_program(loop_n=1):
    if loop_n in _prog_cache:
        return _prog_cache[loop_n]

    nc = bacc.Bacc("TRN2", target_bir_lowering=False, debug=False, num_devices=N_CORES)
    f32 = mybir.dt.float32
    f8 = mybir.dt.float8e4
    x_d = nc.dram_tensor("x2", [P, NCH, 2, XP], f8, kind="ExternalInput")
    w_d = nc.dram_tensor("W", [NG, P, GW], f8, kind="ExternalInput")
    o_d = nc.dram_tensor("out", [2, NCLP], f32, kind="ExternalOutput")

    with tile.TileContext(nc) as tc:
        with (
            tc.tile_pool(name="xpool", bufs=1) as xpool,
            tc.tile_pool(name="wpool", bufs=NG + 1) as wpool,
            tc.tile_pool(name="opool", bufs=2) as opool,
            tc.tile_pool(name="psum", bufs=2, space="PSUM") as psum_pool,
        ):
            x_t = xpool.tile([P, NCH, 2, XP], f8)
            nc.sync.dma_start(x_t[:], x_d[:])

            for _ in range(loop_n):
                ps0 = psum_pool.tile([2, 512], f32, tag="ps0")
                ps1 = psum_pool.tile([2, 512], f32, tag="ps1")
                ps = [ps0, ps1]
                for g in range(NG):
                    w_t = wpool.tile([P, GCH, 2, NCLP], f8, tag="w")
                    nc.sync.dma_start(w_t[:], w_d[g])
                    for jj in range(GCH):
                        j = g * GCH + jj
                        lhsT = x_t[:, j, :, 0:2]
                        for b in range(NB):
                            rhs = w_t[:, jj, :, b * 512 : (b + 1) * 512]
                            nc.tensor.matmul(
                                ps[b][:],
                                lhsT,
                                rhs,
                                start=(j == 0),
                                stop=(j == NCH - 1),
                                perf_mode=mybir.MatmulPerfMode.DoubleRow,
                            )
                ot = opool.tile([2, NCLP], f32, tag="o")
                for b in range(NB):
                    nc.vector.tensor_copy(ot[:, b * 512 : (b + 1) * 512], ps[b][:])
                nc.scalar.dma_start(o_d[:], ot[:])

    nc.finalize()
    _prog_cache[loop_n] = nc
    return nc


def _in_maps(x, W, b):
    x_flat = np.asarray(x, dtype=np.float32).reshape(N_IN)
    W32 = np.asarray(W, dtype=np.float32)
    in_maps = []
    for c in range(N_CORES):
        ks = slice(c * KS, (c + 1) * KS)
        xs = x_flat[ks].reshape(NCH, 2, P)
        hi = xs.astype(F8)
        lo = (xs - hi.astype(np.float32)).astype(F8)
        x4 = np.zeros((P, NCH, 2, XP), dtype=F8)
        x4[:, :, :, 0] = hi.transpose(2, 0, 1)
        x4[:, :, :, 1] = lo.transpose(2, 0, 1)
        # W block: [1000, 18432] -> [NCH, P, 2, 1024] -> grouped [NG, P, GW]
        wc = W32[:, ks].astype(F8)
        a = np.zeros((NCH, P, 2, NCLP), dtype=F8)
        a[:, :, :, :N_CLASSES] = wc.reshape(N_CLASSES, NCH, 2, P).transpose(1, 3, 2, 0)
        wg = np.ascontiguousarray(
            a.reshape(NG, GCH, P, 2, NCLP).transpose(0, 2, 1, 3, 4).reshape(NG, P, GW)
        )
        in_maps.append({"x2": x4, "W": wg})
    return in_maps


def _run(x, W, b, trace=False, loop_n=1, **kwargs):
    nc = _build_program(loop_n)
    in_maps = _in_maps(x, W, b)
    return run_bass_kernel_spmd(nc, in_maps, list(range(N_CORES)), trace=trace, **kwargs)


def kernel(x, W, b):
    res = _run(x, W, b)
    acc = np.zeros(NCLP, dtype=np.float32)
    for r in res.results:
        o = np.asarray(r["out"], dtype=np.float32)
        acc += o[0] + o[1]
    out = acc[:N_CLASSES] + np.asarray(b, dtype=np.float32)
    return out.reshape(1, N_CLASSES).astype(np.float32)


# revision 5
# speedup vs baseline: 1.6110x; 1.1752x over previous
"""GEMV kernel for Trainium2: out = x @ W.T + b, sharded over 8 NeuronCores.

Shapes (hardcoded): x [1, 147456] f32, W [1000, 147456] f32, b [1000] f32.

Sharding: k-parallel (input-dim). Core c takes k-slice [c*18432, (c+1)*18432)
of ALL 1000 classes; each core produces partial dot products over its
k-slice and the host sums the 8 partials (+bias) — a 16K-flop unshard vs
147M device MACs.

Per-core strategy (memory-bound -> minimize HBM bytes; PE does the math):
  - W is ternary {-1,0,1}: exactly representable in fp8e4 (e4m3), quartering
    HBM traffic vs f32 (73.7 MB -> 18.6 MB per core incl. class padding
    1000->1008).
  - x is split x = x_hi + x_lo, both fp8e4 (x_hi = fp8(x), x_lo =
    fp8(x - x_hi)); measured rel err 7.2e-4, far under the 2e-2 gate.
  - PE matmuls run perf_mode=DoubleRow: each cell holds 2 fp8 weights, so
    one matmul contracts 256 k-values and streams W at 2 elem/cycle/
    partition. Per exec: 72 chunks x 2 psum banks = 144 N=504 matmuls
    accumulated in PSUM (start at chunk 0, stop at chunk 71), ~37 us PE —
    under the ~51 us HBM floor (18.6 MB at the measured 364 GB/s per-core
    line rate), so the kernel is DMA-bound.
  - lhsT (stationary x hi/lo pair) is a 3D AP [128, 2, 2], pair dim padded
    to stride 16 (LDWEIGHTS DoubleRow wants step%16==0). rhs (moving W
    block) is [128, 2, 504] with pair stride NCLP.
  - W layout: per 256-k chunk, a [128 part, 2, 1008 cls] fp8 block; chunks
    grouped 6-per-DMA (12 x 1.5 MB line-rate DMAs per exec, alternating
    between the SP and ACT HWDGE rings to overlap per-DMA fixed costs)
    with a 14-slot tile pool so the DMA stream runs ahead of the PE and
    never stalls at exec boundaries.
  - Tail per exec: DVE copies psum[2,504]x2 -> SBUF, out-DMA on the SWDGE
    (gpsimd) path to keep both HWDGE rings free for W loads. Host sums
    hi/lo rows during the cross-core reduce.
  - loop_n repeats the whole computation in-NEFF (each exec re-reads all of
    W from HBM). There is a ~4 ms fixed per-NEFF-call dispatch cost in this
    environment that does NOT overlap device execution, so timing uses a
    large loop_n to amortize it; compile is fast (~5 s per 256 execs).
"""

import numpy as np
import ml_dtypes

import concourse.bacc as bacc
import concourse.mybir as mybir
import concourse.tile as tile
from concourse.bass_utils import run_bass_kernel_spmd

N_CORES = 8
N_CLASSES = 1000
N_IN = 147456
P = 128                       # SBUF partitions
KS = N_IN // N_CORES          # 18432 k values per core
CH = 2 * P                    # 256 k per DoubleRow chunk
NCH = KS // CH                # 72 chunks per core
NCLP = 1008                   # classes padded (63*16 keeps strides 16-aligned)
NH = NCLP // 2                # 504 classes per psum bank
NB = 2                        # psum banks
GCH = 6                       # chunks per DMA group
NG = NCH // GCH               # 12 DMA groups
GW = GCH * 2 * NCLP           # 12096 free bytes per group tile
XP = 16                       # x pair-stride padding (LDW step%16==0)
WBUFS = NG + 2                # W pool slots: stream runs a full group ahead

F8 = ml_dtypes.float8_e4m3

_prog_cache = {}


def _build_program(loop_n=1):
    if loop_n in _prog_cache:
        return _prog_cache[loop_n]

    nc = bacc.Bacc("TRN2", target_bir_lowering=False, debug=False, num_devices=N_CORES)
    f32 = mybir.dt.float32
    f8 = mybir.dt.float8e4
    x_d = nc.dram_tensor("x2", [P, NCH, 2, XP], f8, kind="ExternalInput")
    w_d = nc.dram_tensor("W", [NG, P, GW], f8, kind="ExternalInput")
    o_d = nc.dram_tensor("out", [2, NCLP], f32, kind="ExternalOutput")

    with tile.TileContext(nc) as tc:
        with (
            tc.tile_pool(name="xpool", bufs=1) as xpool,
            tc.tile_pool(name="wpool", bufs=WBUFS) as wpool,
            tc.tile_pool(name="opool", bufs=2) as opool,
            tc.tile_pool(name="psum", bufs=2, space="PSUM") as psum_pool,
        ):
            x_t = xpool.tile([P, NCH, 2, XP], f8)
            nc.sync.dma_start(x_t[:], x_d[:])

            for _ in range(loop_n):
                ps0 = psum_pool.tile([2, NH], f32, tag="ps0")
                ps1 = psum_pool.tile([2, NH], f32, tag="ps1")
                ps = [ps0, ps1]
                for g in range(NG):
                    w_t = wpool.tile([P, GCH, 2, NCLP], f8, tag="w")
                    eng = nc.scalar if g % 2 else nc.sync
                    eng.dma_start(w_t[:], w_d[g])
                    for jj in range(GCH):
                        j = g * GCH + jj
                        lhsT = x_t[:, j, :, 0:2]
                        for b in range(NB):
                            rhs = w_t[:, jj, :, b * NH : (b + 1) * NH]
                            nc.tensor.matmul(
                                ps[b][:],
                                lhsT,
                                rhs,
                                start=(j == 0),
                                stop=(j == NCH - 1),
                                perf_mode=mybir.MatmulPerfMode.DoubleRow,
                            )
                ot = opool.tile([2, NCLP], f32, tag="o")
                for b in range(NB):
                    nc.vector.tensor_copy(ot[:, b * NH : (b + 1) * NH], ps[b][:])
                nc.gpsimd.dma_start(o_d[:], ot[:])

    nc.finalize()
    _prog_cache[loop_n] = nc
    return nc


def _in_maps(x, W, b):
    x_flat = np.asarray(x, dtype=np.float32).reshape(N_IN)
    W32 = np.asarray(W, dtype=np.float32)
    in_maps = []
    for c in range(N_CORES):
        ks = slice(c * KS, (c + 1) * KS)
        xs = x_flat[ks].reshape(NCH, 2, P)
        hi = xs.astype(F8)
        lo = (xs - hi.astype(np.float32)).astype(F8)
        x4 = np.zeros((P, NCH, 2, XP), dtype=F8)
        x4[:, :, :, 0] = hi.transpose(2, 0, 1)
        x4[:, :, :, 1] = lo.transpose(2, 0, 1)
        # W block: [1000, 18432] -> [NCH, P, 2, NCLP] -> grouped [NG, P, GW]
        wc = W32[:, ks].astype(F8)
        a = np.zeros((NCH, P, 2, NCLP), dtype=F8)
        a[:, :, :, :N_CLASSES] = wc.reshape(N_CLASSES, NCH, 2, P).transpose(1, 3, 2, 0)
        wg = np.ascontiguousarray(
            a.reshape(NG, GCH, P, 2, NCLP).transpose(0, 2, 1, 3, 4).reshape(NG, P, GW)
        )
        in_maps.append({"x2": x4, "W": wg})
    return in_maps


def _run(x, W, b, trace=False, loop_n=1, **kwargs):
    nc = _build_program(loop_n)
    in_maps = _in_maps(x, W, b)
    return run_bass_kernel_spmd(nc, in_maps, list(range(N_CORES)), trace=trace, **kwargs)


def kernel(x, W, b):
    res = _run(x, W, b)
    acc = np.zeros(NCLP, dtype=np.float32)
    for r in res.results:
        o = np.asarray(r["out"], dtype=np.float32)
        acc += o[0] + o[1]
    out = acc[:N_CLASSES] + np.asarray(b, dtype=np.float32)
    return out.reshape(1, N_CLASSES).astype(np.float32)
